# revision 25
# baseline (speedup 1.0000x reference)
"""Trainium2 Bass kernel for a 3-layer GCN (Kipf-Welling, symmetric norm,
self-loops) with global add pooling.

Distribution: nodes (graph-aligned contiguous ranges) are sharded across 8
NeuronCores.  Each core owns the aggregation (scatter-add) for its local dst
nodes.  Per layer the (dis-scaled) projected features Hs are exchanged in 4
tile-BANDS via 4 pipelined AllGathers; band j's edge gathers run on SWDGE
queue j so desc-gen pipelines across the 4 Q7 cpu pairs.

Math (matches the jax reference exactly):
    deg  = indeg + 1, dis = deg^-1/2
    Hsf  = dis * (H @ W)                      (feature-major, kept in SBUF)
    agg_d = dis_d * (sum_{e:(s->d)} Hsf_s  +  Hsf_d) + b     (self loop local)
    H'   = relu(agg)                          (no relu on layer 3)
    out  = segment_sum(H3, batch)

The edge scatter-add is a matmul with an on-the-fly selection matrix
S[e, d] = (dst_id[e] == d) in fp16; gathered source rows are fp16.
Layer 3 is zero-padded to 128 output features so fp16 table rows stay 256B
(dma_gather elem_size restriction).
"""

import os
import sys
import math

import numpy as np

sys.path.insert(0, "/opt/trn_rl_repo")

import concourse.bass as bass  # noqa: E402
import concourse.bacc as bacc  # noqa: E402
import concourse.tile as tile  # noqa: E402
from concourse import mybir  # noqa: E402
from concourse.bass_utils import run_bass_kernel_spmd  # noqa: E402
from concourse.masks import make_identity  # noqa: E402

P = 128
F32 = mybir.dt.float32
F16 = mybir.dt.float16
I32 = mybir.dt.int32
I16 = mybir.dt.int16
OP = mybir.AluOpType
AF = mybir.ActivationFunctionType

N_CORES = 8
G_TOTAL = 1000  # graphs in the batch (fixed by the problem)
NB_BANDS = 4    # src-tile bands (= SWDGE queues = pipelined allgathers)
TB = 2          # dst tiles per gather block


# ----------------------------------------------------------------------------
# Host-side preprocessing: shard nodes/edges, build gather/selection metadata.
# ----------------------------------------------------------------------------

def _preprocess(x, edge_index, batch, n_cores, G):
    N = x.shape[0]
    src = edge_index[0].astype(np.int64)
    dst = edge_index[1].astype(np.int64)
    batch = batch.astype(np.int64)

    # graph-aligned shard boundaries near equal node counts
    graph_start = np.searchsorted(batch, np.arange(G + 1))  # [G+1], node idx
    bounds = [0]
    for c in range(1, n_cores):
        target = (c * N) // n_cores
        gi = np.searchsorted(graph_start, target)
        lo = graph_start[gi - 1] if gi > 0 else 0
        hi = graph_start[gi] if gi <= G else N
        b = int(hi if (hi - target) <= (target - lo) else lo)
        b = max(b, bounds[-1])  # keep non-decreasing
        bounds.append(b)
    bounds.append(N)
    bounds = np.asarray(bounds, dtype=np.int64)

    shard_sizes = bounds[1:] - bounds[:-1]
    N_loc = int(math.ceil(int(shard_sizes.max()) / P) * P)
    T = N_loc // P
    NB = (T + TB - 1) // TB

    # band boundaries over local tiles
    bs = (T + NB_BANDS - 1) // NB_BANDS
    band_lo = [min(j * bs, T) for j in range(NB_BANDS + 1)]
    band_tiles = [band_lo[j + 1] - band_lo[j] for j in range(NB_BANDS)]

    # normalization (index-derived scalars)
    deg = np.bincount(dst, minlength=N).astype(np.float32) + np.float32(1.0)
    dis = (np.float32(1.0) / np.sqrt(deg)).astype(np.float32)

    # src row id within its band table:
    # band table j rows: (src_core * band_tiles[j] + tile_in_band)*128 + pos
    core_of = np.searchsorted(bounds, dst, side="right") - 1
    core_of_src = np.searchsorted(bounds, src, side="right") - 1
    src_loc = src - bounds[core_of_src]
    src_tile = src_loc // P
    src_band = np.minimum(src_tile // bs, NB_BANDS - 1)
    src_row = (core_of_src * np.asarray(band_tiles)[src_band]
               + (src_tile - np.asarray(band_lo)[src_band])) * P + src_loc % P
    assert int(src_row.max(initial=0)) < 32768

    # ---- per-core edge bucketing by (dst block, src band), sorted by dst
    per_core = []
    for c in range(n_cores):
        m = core_of == c
        dstl = dst[m] - bounds[c]
        rows = src_row[m]
        bands = src_band[m]
        blk = dstl // (TB * P)
        key = blk * NB_BANDS + bands
        order = np.lexsort((dstl, key))
        dstl, rows, key = dstl[order], rows[order], key[order]
        cnt = np.bincount(key, minlength=NB * NB_BANDS)
        per_core.append((dstl, rows, key, cnt))

    cnts = np.stack([pc[3] for pc in per_core])        # [cores, NB*NB_BANDS]
    CH = np.ceil(cnts.max(axis=0) / P).astype(np.int64)  # chunks per group
    ch_off = np.concatenate([[0], np.cumsum(CH)])      # chunk col offsets
    total_ch = int(ch_off[-1])
    # idx16 column offset per group (16-wrapped, so 8 cols per chunk)
    col_off = ch_off * (P // 16)

    # per-(group, tile-in-block) chunk subranges, unioned over cores
    rng_lo = np.full((NB * NB_BANDS, TB), 10 ** 9, dtype=np.int64)
    rng_hi = np.full((NB * NB_BANDS, TB), -1, dtype=np.int64)

    in_maps = []
    g_lo = []
    g_cnt = []
    for c in range(n_cores):
        dstl, rows, key, cnt = per_core[c]
        goff = np.concatenate([[0], np.cumsum(cnt)])[:-1]
        slot = (np.arange(dstl.shape[0]) - goff[key]) + ch_off[key] * P

        # trailing -1 pads are trimmed by the Q7 firmware; num_idxs_reg
        # carries the matching per-core count so the decode-side ring
        # reservation agrees with what the firmware pushes.
        idx16 = np.full((16, total_ch * P // 16), -1, dtype=np.int16)
        idx16[slot % 16, slot // 16] = rows.astype(np.int16)
        srcidx = np.tile(idx16, (8, 1))

        dstid = np.full((P, total_ch), 30000.0, dtype=np.float16)
        dstid[slot % P, slot // P] = (dstl % (TB * P)).astype(np.float32)

        # per-core tile chunk ranges -> union
        tl = (dstl // P) % TB
        gkey = key * TB + tl
        gcnt = np.bincount(gkey, minlength=NB * NB_BANDS * TB)
        goff2 = np.concatenate([[0], np.cumsum(gcnt)])
        for g in range(NB * NB_BANDS):
            for t2 in range(TB):
                s0, s1 = goff2[g * TB + t2], goff2[g * TB + t2 + 1]
                if s1 > s0:
                    a = (s0 - goff[g]) // P
                    b = (s1 - 1 - goff[g]) // P + 1
                    rng_lo[g, t2] = min(rng_lo[g, t2], a)
                    rng_hi[g, t2] = max(rng_hi[g, t2], b)

        n_real = int(bounds[c + 1] - bounds[c])
        dis_loc = np.ones(N_loc, dtype=np.float32)
        dis_loc[:n_real] = dis[bounds[c]:bounds[c + 1]]
        disrep = np.broadcast_to(dis_loc.astype(np.float16), (P, N_loc)).copy()

        xT = np.zeros((P, N_loc), dtype=np.float32)
        xT[:, :n_real] = x[bounds[c]:bounds[c + 1]].T

        bloc = batch[bounds[c]:bounds[c + 1]]
        glo = int(bloc[0]) if n_real > 0 else 0
        gct = int(bloc[-1]) + 1 - glo if n_real > 0 else 0
        g_lo.append(glo)
        g_cnt.append(gct)
        gcnt = cnt.astype(np.int32).reshape(1, -1)
        in_maps.append(dict(srcidx=srcidx, dstid=dstid, disrep=disrep, xT=xT,
                            gcnt=gcnt, _bloc=bloc - glo, _n_real=n_real))

    GW = max(1, int(math.ceil(max(g_cnt) / P)))
    iota2 = np.zeros((P, TB * P), dtype=np.float16)
    for t2 in range(TB):
        iota2[:, t2 * P:(t2 + 1) * P] = np.arange(P) + t2 * P
    for c in range(n_cores):
        d = in_maps[c]
        bloc, n_real = d.pop("_bloc"), d.pop("_n_real")
        poolid = np.full((P, T * GW), 30000.0, dtype=np.float16)
        j = np.arange(n_real)
        for w in range(GW):
            poolid[j % P, (j // P) + w * T] = (bloc - w * P).astype(np.float32)
        d["poolid"] = poolid
        d["iota2"] = iota2

    rngs = np.where(rng_hi < 0, 0, rng_hi - np.maximum(rng_lo, 0))
    cfg = dict(T=T, NB=NB, GW=GW, N_loc=N_loc, n_cores=n_cores,
               band_tiles=band_tiles, band_lo=band_lo,
               CH=CH.tolist(), ch_off=ch_off.tolist(),
               col_off=col_off.tolist(),
               rng_lo=np.maximum(rng_lo, 0).tolist(),
               rng_hi=np.maximum(rng_hi, 0).tolist(),
               max_rng=int(rngs.max()), max_ch=int(CH.max()))
    return cfg, in_maps, bounds, g_lo, g_cnt


# ----------------------------------------------------------------------------
# Bass program
# ----------------------------------------------------------------------------

def _build_program(cfg):
    T, NB, GW, N_loc = cfg["T"], cfg["NB"], cfg["GW"], cfg["N_loc"]
    n_cores = cfg["n_cores"]
    band_tiles, band_lo = cfg["band_tiles"], cfg["band_lo"]
    CH, ch_off, col_off = cfg["CH"], cfg["ch_off"], cfg["col_off"]
    rng_lo, rng_hi = cfg["rng_lo"], cfg["rng_hi"]
    total_ch = ch_off[-1]
    D, DO = 128, 64

    nc = bacc.Bacc(None, num_devices=n_cores, num_swdge_queues=4)

    xT_d = nc.dram_tensor("xT", [P, N_loc], F32, kind="ExternalInput")
    W_d = {l: nc.dram_tensor(f"W{l}", [D, D], F32, kind="ExternalInput")
           for l in range(4)}
    b_d = {l: nc.dram_tensor(f"b{l}", [P, 1], F32, kind="ExternalInput")
           for l in range(4)}
    srcidx_d = nc.dram_tensor("srcidx", [P, total_ch * P // 16], I16,
                              kind="ExternalInput")
    dstid_d = nc.dram_tensor("dstid", [P, total_ch], F16,
                             kind="ExternalInput")
    disrep_d = nc.dram_tensor("disrep", [P, N_loc], F16, kind="ExternalInput")
    poolid_d = nc.dram_tensor("poolid", [P, T * GW], F16, kind="ExternalInput")
    iota2_d = nc.dram_tensor("iota2", [P, TB * P], F16, kind="ExternalInput")
    n_groups = NB * NB_BANDS
    gcnt_d = nc.dram_tensor("gcnt", [1, n_groups], I32, kind="ExternalInput")
    out_d = nc.dram_tensor("out", [GW * P, DO], F32, kind="ExternalOutput")
    dbg_stage = os.environ.get("GCN_DBG_STAGE", "")
    dbg_d = None
    if dbg_stage.startswith("h"):
        dbg_d = nc.dram_tensor("dbg", [P, N_loc], F32, kind="ExternalOutput")

    with tile.TileContext(nc) as tc:
        with tc.tile_pool(name="const", bufs=1) as const, \
             tc.tile_pool(name="hpool", bufs=1) as hpool, \
             tc.tile_pool(name="stage", bufs=3) as stage, \
             tc.tile_pool(name="rpool", bufs=10) as rpool, \
             tc.tile_pool(name="spool", bufs=2) as spool, \
             tc.tile_pool(name="dram", bufs=2, space="DRAM") as dram, \
             tc.tile_pool(name="pm", bufs=2, space="PSUM") as pm, \
             tc.tile_pool(name="pq", bufs=1, space="PSUM") as pq, \
             tc.tile_pool(name="pt", bufs=2, space="PSUM") as pt, \
             tc.tile_pool(name="pa", bufs=2, space="PSUM") as pa:

            # ---- constants into SBUF
            w_sb = {}
            for l in range(4):
                w = const.tile([D, D], F32, name=f"w{l}sb")
                nc.sync.dma_start(out=w[:], in_=W_d[l][:, :])
                w_sb[l] = w
            b_sb = {}
            for l in range(4):
                b = const.tile([P, 1], F32, name=f"b{l}sb")
                nc.sync.dma_start(out=b[:], in_=b_d[l][:, :])
                b_sb[l] = b
            w16_sb = {}
            for l in range(1, 4):
                w16 = const.tile([D, D], F16, name=f"w16{l}sb")
                nc.scalar.copy(out=w16[:], in_=w_sb[l][:])
                w16_sb[l] = w16
            iota2_sb = const.tile([P, TB * P], F16, name="iota2sb")
            nc.sync.dma_start(out=iota2_sb[:], in_=iota2_d[:, :])
            iden16_sb = const.tile([P, P], F16, name="iden16sb")
            make_identity(nc, iden16_sb[:])
            srcidx_sb = const.tile([P, total_ch * P // 16], I16,
                                   name="srcidxsb")
            nc.sync.dma_start(out=srcidx_sb[:], in_=srcidx_d[:, :])
            dstid_sb = const.tile([P, total_ch], F16, name="dstidsb")
            disrep_sb = const.tile([P, N_loc], F16, name="disrepsb")
            poolid_sb = const.tile([P, T * GW], F16, name="poolidsb")
            nc.sync.dma_start(out=dstid_sb[:], in_=dstid_d[:, :])
            nc.sync.dma_start(out=disrep_sb[:], in_=disrep_d[:, :])
            nc.sync.dma_start(out=poolid_sb[:], in_=poolid_d[:, :])
            gcnt_sb = const.tile([1, n_groups], I32, name="gcntsb")
            nc.sync.dma_start(out=gcnt_sb[:], in_=gcnt_d[:, :])

            H = hpool.tile([P, N_loc], F16, name="H", tag="ha")
            Hsf = hpool.tile([P, N_loc], F16, name="Hsf", tag="hsf")

            # Zero the R pool once: trimmed (-1) gather slots are never
            # written by the DMA, and a NaN in untouched SBUF would poison
            # the scatter matmul (0 * NaN).  After this, stale slots only
            # ever hold old finite Hs values.
            for _ in range(10):
                R0 = rpool.tile([P, cfg["max_ch"] * D], F16,
                                name="R", tag="R")
                nc.vector.memset(R0[:, :], 0.0)

            # Join const-load DMA sems into the DVE engine clock so later DVE
            # tensor_tensor ops don't carry per-DMA waits themselves.
            joiner = const.tile([P, 1], F32, name="joiner")
            for cst in [iota2_sb, dstid_sb, disrep_sb, poolid_sb,
                        b_sb[0], b_sb[1], b_sb[2], b_sb[3]]:
                nc.vector.tensor_copy(out=joiner[:, :1], in_=cst[:, :1])

            NKS = (N_loc + 511) // 512  # phase1 strips

            # ---- phase 1 strip: project cols [512k, 512k+512) of layer l.
            #      l=0 streams xT -> H; l>=1: Hsf = dis * (W^T H), transpose
            #      tiles into fp16 band tables.
            def phase1_strip(l, k, HsBands):
                c0 = k * 512
                cw = min(512, N_loc - c0)
                if l == 0:
                    xst = stage.tile([P, 512], F32, name="xst", tag="ms")
                    nc.sync.dma_start(out=xst[:, :cw],
                                      in_=xT_d[:, c0:c0 + cw])
                    mm = pm.tile([P, 512], F32, name="mm", tag="pm")
                    nc.tensor.matmul(mm[:, :cw], lhsT=w_sb[0][:, :],
                                     rhs=xst[:, :cw], start=True, stop=True)
                    nc.scalar.activation(
                        out=H[:, c0:c0 + cw], in_=mm[:, :cw],
                        func=AF.Identity, bias=b_sb[0][:, :], scale=1.0)
                    return
                mm = pm.tile([P, 512], F32, name="mm", tag="pm")
                nc.tensor.matmul(mm[:, :cw], lhsT=w16_sb[l][:, :],
                                 rhs=H[:, c0:c0 + cw], start=True, stop=True)
                nc.vector.tensor_tensor(
                    out=Hsf[:, c0:c0 + cw], in0=mm[:, :cw],
                    in1=disrep_sb[:, c0:c0 + cw], op=OP.mult)
                for tt in range(cw // P):
                    t = k * 4 + tt
                    j = 0
                    while t >= band_lo[j + 1]:
                        j += 1
                    trow = (t - band_lo[j]) * P
                    ptt = pt.tile([P, P], F16, name="ptt", tag="pt")
                    nc.tensor.transpose(
                        out=ptt[:, :],
                        in_=Hsf[:, t * P:(t + 1) * P],
                        identity=iden16_sb[:, :])
                    hs = stage.tile([P, P], F16, name="hs", tag="hs")
                    nc.scalar.copy(out=hs[:, :], in_=ptt[:, :])
                    nc.sync.dma_start(
                        out=HsBands[j][trow:trow + P, :],
                        in_=hs[:, :])

            def emit_cc(l, j, HsBands, HsFullBands):
                no_cc = int(os.environ.get("GCN_NO_CC", "0"))
                HsFull = dram.tile([n_cores * band_tiles[j] * P, D], F16,
                                   name=f"hsf{l}_{j}", tag=f"hsf{j}",
                                   addr_space="Local" if no_cc else "Shared")
                if no_cc:
                    nb_rows = band_tiles[j] * P
                    for cc_i in range(n_cores):
                        nc.sync.dma_start(
                            out=HsFull[cc_i * nb_rows:
                                       (cc_i + 1) * nb_rows, :],
                            in_=HsBands[j][:, :])
                else:
                    nc.gpsimd.collective_compute(
                        "AllGather", OP.bypass,
                        replica_groups=[list(range(n_cores))],
                        ins=[HsBands[j][:, :].opt()],
                        outs=[HsFull[:, :].opt()])
                HsFullBands.append(HsFull)

            def make_bands(l):
                return [dram.tile([band_tiles[j] * P, D], F16,
                                  name=f"hsl{l}_{j}", tag=f"hsl{j}")
                        for j in range(NB_BANDS)]

            # CC_j of the next layer fires after this phase1 strip:
            cc_strip = [(band_lo[j + 1] - 1) // 4 for j in range(NB_BANDS)]
            # phase1 strip s of the next layer is emitted after this block:
            strip_block = [min(2 * s + 1, NB - 1) for s in range(NKS)]

            # ---- phase 2: per block, gather 4 band groups (queues 0-3),
            #      then per tile scatter-matmul + local self term.  The next
            #      layer's phase1 strips + allgathers are interleaved so they
            #      hide under this layer's gather stream.
            DELAY = 2  # blocks the j=3 gathers lag behind j<3

            def gather_group(l, tb, j, HsFullBands):
                g = tb * NB_BANDS + j
                nch = CH[g]
                if nch == 0:
                    return None
                R = rpool.tile([P, cfg["max_ch"] * D], F16,
                               name="R", tag="R")
                num = nch * P
                gcnt_reg = nc.gpsimd.alloc_register(f"gc{l}_{g}")
                nc.gpsimd.reg_load(gcnt_reg, gcnt_sb[0:1, g:g + 1])
                nc.gpsimd.dma_gather(
                    out_ap=R[:, :nch * D].rearrange("p (c e) -> p c e", e=D),
                    in_ap=HsFullBands[j][:, :],
                    idxs_ap=srcidx_sb[:, col_off[g]:col_off[g] + num // 16],
                    num_idxs=num,
                    num_idxs_reg=gcnt_reg,
                    elem_size=D,
                    single_packet=False,
                    queue_num=j)
                return R

            # ---- phase 2: j<3 band gathers stream ahead; this layer's last
            #      allgather (band 3) is emitted after DELAY blocks so its
            #      table-wait never stalls the gather queue; j=3 gathers and
            #      the scatter/finalize lag DELAY blocks behind.  The next
            #      layer's phase1 strips + first 3 allgathers interleave here.
            def phase2(l, own, H3, nxt):
                pend = {}
                for it in range(NB + DELAY):
                    if it < NB:
                        pend[it] = [gather_group(l, it, j, own[1])
                                    for j in range(NB_BANDS - 1)]
                    if it == DELAY - 1:
                        emit_cc(l, NB_BANDS - 1, own[0], own[1])
                    if it >= DELAY:
                        tb = it - DELAY
                        Rb = pend.pop(tb)
                        Rb.append(gather_group(l, tb, NB_BANDS - 1, own[1]))
                        tbg = min(TB, T - tb * TB)
                        for tl in range(tbg):
                            phase2_tile(l, tb, tl, Rb, H3)
                        if nxt is not None:
                            for s2 in range(NKS):
                                if strip_block[s2] == tb:
                                    phase1_strip(l + 1, s2, nxt[0])
                                    for j in range(NB_BANDS - 1):
                                        if cc_strip[j] == s2:
                                            emit_cc(l + 1, j, nxt[0], nxt[1])

            def phase2_tile(l, tb, tl, Rb, H3):
                t = tb * TB + tl
                # selection matrices + accumulate matmuls over 4 bands
                mms = []
                for j in range(NB_BANDS):
                    g = tb * NB_BANDS + j
                    a, b2 = rng_lo[g][tl], rng_hi[g][tl]
                    if b2 <= a or Rb[j] is None:
                        continue
                    nr = b2 - a
                    S = spool.tile([P, cfg["max_rng"] * P], F16,
                                   name="S", tag="S")
                    nc.vector.tensor_tensor(
                        out=S[:, :nr * P].rearrange("p (c d) -> p c d", d=P),
                        in0=dstid_sb[:, ch_off[g] + a:ch_off[g] + b2]
                            .unsqueeze(2).broadcast_to([P, nr, P]),
                        in1=iota2_sb[:, tl * P:(tl + 1) * P]
                            .unsqueeze(1).broadcast_to([P, nr, P]),
                        op=OP.is_equal)
                    for cc in range(a, b2):
                        mms.append((Rb[j], cc, S, cc - a))
                agg = pa.tile([P, P], F32, name="agg", tag="pa")
                if not mms:
                    nc.vector.memset(agg[:, :], 0.0)
                for k, (R, cc, S, sc) in enumerate(mms):
                    nc.tensor.matmul(
                        agg[:, :],
                        lhsT=R[:, cc * D:(cc + 1) * D],
                        rhs=S[:, sc * P:(sc + 1) * P],
                        start=(k == 0), stop=(k == len(mms) - 1))
                # out = dis_d * (agg + Hsf_d) + b ; relu except l=3
                tmp = stage.tile([P, P], F32, name="tmp", tag="tmp")
                nc.vector.tensor_tensor(
                    out=tmp[:, :], in0=agg[:, :],
                    in1=Hsf[:, t * P:(t + 1) * P], op=OP.add)
                nc.vector.tensor_tensor(
                    out=tmp[:, :], in0=tmp[:, :],
                    in1=disrep_sb[:, t * P:(t + 1) * P], op=OP.mult)
                if l < 3:
                    nc.scalar.activation(
                        out=H[:, t * P:(t + 1) * P], in_=tmp[:, :],
                        func=AF.Relu, bias=b_sb[l][:, :], scale=1.0)
                else:
                    t2 = stage.tile([P, P], F16, name="t2", tag="tmp2")
                    nc.scalar.activation(
                        out=t2[:, :], in_=tmp[:, :],
                        func=AF.Identity, bias=b_sb[3][:, :], scale=1.0)
                    ptt = pt.tile([P, P], F16, name="ptt2", tag="pt")
                    nc.tensor.transpose(
                        out=ptt[:, :], in_=t2[:, :],
                        identity=iden16_sb[:, :])
                    nc.scalar.copy(
                        out=H3[:, t * DO:(t + 1) * DO],
                        in_=ptt[:, :DO])
                    if GW == 1:
                        sp = spool.tile([P, P], F16, name="sp", tag="sp")
                        nc.vector.tensor_tensor(
                            out=sp[:],
                            in0=poolid_sb[:, t:t + 1].to_broadcast([P, P]),
                            in1=iota2_sb[:, :P], op=OP.is_equal)
                        nc.tensor.matmul(pp_hold[0][:], lhsT=sp[:],
                                         rhs=H3[:, t * DO:(t + 1) * DO],
                                         start=(t == 0), stop=(t == T - 1))

            def dump_dbg(buf, width=None):
                nc.sync.dma_start(
                    out=dbg_d[:, :width] if width else dbg_d[:, :],
                    in_=buf[:, :width] if width else buf[:, :])

            # ---- the network
            H3 = hpool.tile([P, T * DO], F16, name="H3", tag="hx")
            bands1 = make_bands(1)
            full1 = []
            for k in range(NKS):
                phase1_strip(0, k, None)  # embedding -> H (streams xT)
                phase1_strip(1, k, bands1)
                for j in range(NB_BANDS - 1):
                    if cc_strip[j] == k:
                        emit_cc(1, j, bands1, full1)
            if dbg_stage == "h0":
                dump_dbg(H)
            ctx = (bands1, full1)
            pp_hold = []
            for l in (1, 2, 3):
                if l == 3 and GW == 1:
                    pp_hold.append(pq.tile([P, DO], F32, name="pp", tag="pp"))
                if l < 3:
                    nxt = (make_bands(l + 1), [])
                else:
                    nxt = None
                phase2(l, ctx, H3, nxt)
                if dbg_stage == f"h{l}":
                    dump_dbg(H if l < 3 else H3, None if l < 3 else T * DO)
                ctx = nxt

            # ---- global add pool (accumulated inline during layer 3 when
            #      GW == 1; fall back to a tail loop otherwise)
            if GW == 1:
                ost = stage.tile([P, DO], F32, name="ost", tag="ost")
                nc.scalar.copy(out=ost[:], in_=pp_hold[0][:])
                nc.sync.dma_start(out=out_d[0:P, :], in_=ost[:])
            else:
                for w in range(GW):
                    pp = pt.tile([P, DO], F32, name="pp", tag="pp")
                    for t in range(T):
                        sp = spool.tile([P, P], F16, name="sp", tag="sp")
                        nc.vector.tensor_tensor(
                            out=sp[:],
                            in0=poolid_sb[:, w * T + t:w * T + t + 1]
                                .to_broadcast([P, P]),
                            in1=iota2_sb[:, :P], op=OP.is_equal)
                        nc.tensor.matmul(pp[:], lhsT=sp[:],
                                         rhs=H3[:, t * DO:(t + 1) * DO],
                                         start=(t == 0), stop=(t == T - 1))
                    ost = stage.tile([P, DO], F32, name="ost", tag="ost")
                    nc.scalar.copy(out=ost[:], in_=pp[:])
                    nc.sync.dma_start(out=out_d[w * P:(w + 1) * P, :],
                                      in_=ost[:])

    return nc


# ----------------------------------------------------------------------------
# Driver
# ----------------------------------------------------------------------------

def _run(x, edge_index, batch, W_emb, b_emb, W1, b1, W2, b2, W3, b3,
         G=G_TOTAL, n_cores=N_CORES, trace=False):
    x = np.ascontiguousarray(np.asarray(x, dtype=np.float32))
    edge_index = np.ascontiguousarray(np.asarray(edge_index, dtype=np.int64))
    batch_np = np.ascontiguousarray(np.asarray(batch, dtype=np.int64))

    cfg, in_maps, bounds, g_lo, g_cnt = _preprocess(
        x, edge_index, batch_np, n_cores, G)

    def bpad(b):
        v = np.zeros((P, 1), dtype=np.float32)
        b = np.asarray(b, dtype=np.float32).reshape(-1)
        v[:b.shape[0], 0] = b
        return v

    W3p = np.zeros((128, 128), dtype=np.float32)
    W3p[:, :np.asarray(W3).shape[1]] = np.asarray(W3, dtype=np.float32)
    shared = dict(
        W0=np.asarray(W_emb, dtype=np.float32),
        W1=np.asarray(W1, dtype=np.float32),
        W2=np.asarray(W2, dtype=np.float32),
        W3=W3p,
        b0=bpad(b_emb), b1=bpad(b1), b2=bpad(b2), b3=bpad(b3))
    for m in in_maps:
        m.update(shared)

    nc = _build_program(cfg)
    nc.finalize()
    res = run_bass_kernel_spmd(nc, in_maps, list(range(n_cores)),
                               trace=trace)

    out = np.zeros((G, 64), dtype=np.float32)
    for c in range(n_cores):
        oc = np.asarray(res.results[c]["out"])
        if g_cnt[c] > 0:
            out[g_lo[c]:g_lo[c] + g_cnt[c]] = oc[:g_cnt[c]]
    return out, res


def kernel(**inputs):
    out, _ = _run(G=G_TOTAL, n_cores=N_CORES,
                  trace=bool(int(os.environ.get("GCN_TRACE", "0"))),
                  **inputs)
    return out


# revision 26
# speedup vs baseline: 1.7047x; 1.7047x over previous
"""Trainium2 Bass kernel for a 3-layer GCN (Kipf-Welling, symmetric norm,
self-loops) with global add pooling.

Distribution: nodes (graph-aligned contiguous ranges) are sharded across 8
NeuronCores.  Each core owns the aggregation (scatter-add) for its local dst
nodes.  Per layer the (dis-scaled) projected features Hs are exchanged in 4
tile-BANDS via 4 pipelined AllGathers; band j's edge gathers run on SWDGE
queue j so desc-gen pipelines across the 4 Q7 cpu pairs.

Math (matches the jax reference exactly):
    deg  = indeg + 1, dis = deg^-1/2
    Hsf  = dis * (H @ W)                      (feature-major, kept in SBUF)
    agg_d = dis_d * (sum_{e:(s->d)} Hsf_s  +  Hsf_d) + b     (self loop local)
    H'   = relu(agg)                          (no relu on layer 3)
    out  = segment_sum(H3, batch)

The edge scatter-add is a matmul with an on-the-fly selection matrix
S[e, d] = (dst_id[e] == d) in fp16; gathered source rows are fp16.
Layer 3 is zero-padded to 128 output features so fp16 table rows stay 256B
(dma_gather elem_size restriction).
"""

import os
import sys
import math

import numpy as np

sys.path.insert(0, "/opt/trn_rl_repo")

import concourse.bass as bass  # noqa: E402
import concourse.bacc as bacc  # noqa: E402
import concourse.tile as tile  # noqa: E402
from concourse import mybir  # noqa: E402
from concourse.bass_utils import run_bass_kernel_spmd  # noqa: E402
from concourse.masks import make_identity  # noqa: E402

P = 128
F32 = mybir.dt.float32
F16 = mybir.dt.float16
I32 = mybir.dt.int32
I16 = mybir.dt.int16
OP = mybir.AluOpType
AF = mybir.ActivationFunctionType

N_CORES = 8
G_TOTAL = 1000  # graphs in the batch (fixed by the problem)
NB_BANDS = 4    # src-tile bands (= SWDGE queues = pipelined allgathers)
TB = 2          # dst tiles per gather block


# ----------------------------------------------------------------------------
# Host-side preprocessing: shard nodes/edges, build gather/selection metadata.
# ----------------------------------------------------------------------------

def _preprocess(x, edge_index, batch, n_cores, G):
    N = x.shape[0]
    src = edge_index[0].astype(np.int64)
    dst = edge_index[1].astype(np.int64)
    batch = batch.astype(np.int64)

    # graph-aligned shard boundaries near equal node counts
    graph_start = np.searchsorted(batch, np.arange(G + 1))  # [G+1], node idx
    bounds = [0]
    for c in range(1, n_cores):
        target = (c * N) // n_cores
        gi = np.searchsorted(graph_start, target)
        lo = graph_start[gi - 1] if gi > 0 else 0
        hi = graph_start[gi] if gi <= G else N
        b = int(hi if (hi - target) <= (target - lo) else lo)
        b = max(b, bounds[-1])  # keep non-decreasing
        bounds.append(b)
    bounds.append(N)
    bounds = np.asarray(bounds, dtype=np.int64)

    shard_sizes = bounds[1:] - bounds[:-1]
    N_loc = int(math.ceil(int(shard_sizes.max()) / P) * P)
    T = N_loc // P
    NB = (T + TB - 1) // TB

    # band boundaries over local tiles
    bs = (T + NB_BANDS - 1) // NB_BANDS
    band_lo = [min(j * bs, T) for j in range(NB_BANDS + 1)]
    band_tiles = [band_lo[j + 1] - band_lo[j] for j in range(NB_BANDS)]

    # normalization (index-derived scalars)
    deg = np.bincount(dst, minlength=N).astype(np.float32) + np.float32(1.0)
    dis = (np.float32(1.0) / np.sqrt(deg)).astype(np.float32)

    # src row id within its band table:
    # band table j rows: (src_core * band_tiles[j] + tile_in_band)*128 + pos
    core_of = np.searchsorted(bounds, dst, side="right") - 1
    core_of_src = np.searchsorted(bounds, src, side="right") - 1
    src_loc = src - bounds[core_of_src]
    src_tile = src_loc // P
    src_band = np.minimum(src_tile // bs, NB_BANDS - 1)
    src_row = (core_of_src * np.asarray(band_tiles)[src_band]
               + (src_tile - np.asarray(band_lo)[src_band])) * P + src_loc % P
    assert int(src_row.max(initial=0)) < 32768

    # ---- per-core edge bucketing by (dst block, src band), sorted by dst
    per_core = []
    for c in range(n_cores):
        m = core_of == c
        dstl = dst[m] - bounds[c]
        rows = src_row[m]
        bands = src_band[m]
        blk = dstl // (TB * P)
        key = blk * NB_BANDS + bands
        order = np.lexsort((dstl, key))
        dstl, rows, key = dstl[order], rows[order], key[order]
        cnt = np.bincount(key, minlength=NB * NB_BANDS)
        per_core.append((dstl, rows, key, cnt))

    cnts = np.stack([pc[3] for pc in per_core])        # [cores, NB*NB_BANDS]
    CH = np.ceil(cnts.max(axis=0) / P).astype(np.int64)  # chunks per group
    ch_off = np.concatenate([[0], np.cumsum(CH)])      # chunk col offsets
    total_ch = int(ch_off[-1])
    # idx16 column offset per group (16-wrapped, so 8 cols per chunk)
    col_off = ch_off * (P // 16)

    # per-(group, tile-in-block) chunk subranges, unioned over cores
    rng_lo = np.full((NB * NB_BANDS, TB), 10 ** 9, dtype=np.int64)
    rng_hi = np.full((NB * NB_BANDS, TB), -1, dtype=np.int64)

    in_maps = []
    g_lo = []
    g_cnt = []
    for c in range(n_cores):
        dstl, rows, key, cnt = per_core[c]
        goff = np.concatenate([[0], np.cumsum(cnt)])[:-1]
        slot = (np.arange(dstl.shape[0]) - goff[key]) + ch_off[key] * P

        # trailing -1 pads are trimmed by the Q7 firmware; num_idxs_reg
        # carries the matching per-core count so the decode-side ring
        # reservation agrees with what the firmware pushes.
        idx16 = np.full((16, total_ch * P // 16), -1, dtype=np.int16)
        idx16[slot % 16, slot // 16] = rows.astype(np.int16)
        srcidx = np.tile(idx16, (8, 1))

        dstid = np.full((P, total_ch), 30000.0, dtype=np.float16)
        dstid[slot % P, slot // P] = (dstl % (TB * P)).astype(np.float32)

        # per-core tile chunk ranges -> union
        tl = (dstl // P) % TB
        gkey = key * TB + tl
        gcnt = np.bincount(gkey, minlength=NB * NB_BANDS * TB)
        goff2 = np.concatenate([[0], np.cumsum(gcnt)])
        for g in range(NB * NB_BANDS):
            for t2 in range(TB):
                s0, s1 = goff2[g * TB + t2], goff2[g * TB + t2 + 1]
                if s1 > s0:
                    a = (s0 - goff[g]) // P
                    b = (s1 - 1 - goff[g]) // P + 1
                    rng_lo[g, t2] = min(rng_lo[g, t2], a)
                    rng_hi[g, t2] = max(rng_hi[g, t2], b)

        n_real = int(bounds[c + 1] - bounds[c])
        dis_loc = np.ones(N_loc, dtype=np.float32)
        dis_loc[:n_real] = dis[bounds[c]:bounds[c + 1]]
        disrep = np.broadcast_to(dis_loc.astype(np.float16), (P, N_loc)).copy()

        xT = np.zeros((P, N_loc), dtype=np.float32)
        xT[:, :n_real] = x[bounds[c]:bounds[c + 1]].T

        bloc = batch[bounds[c]:bounds[c + 1]]
        glo = int(bloc[0]) if n_real > 0 else 0
        gct = int(bloc[-1]) + 1 - glo if n_real > 0 else 0
        g_lo.append(glo)
        g_cnt.append(gct)
        gcnt = cnt.astype(np.int32).reshape(1, -1)
        in_maps.append(dict(srcidx=srcidx, dstid=dstid, disrep=disrep, xT=xT,
                            gcnt=gcnt, _bloc=bloc - glo, _n_real=n_real))

    GW = max(1, int(math.ceil(max(g_cnt) / P)))
    iota2 = np.zeros((P, TB * P), dtype=np.float16)
    for t2 in range(TB):
        iota2[:, t2 * P:(t2 + 1) * P] = np.arange(P) + t2 * P
    for c in range(n_cores):
        d = in_maps[c]
        bloc, n_real = d.pop("_bloc"), d.pop("_n_real")
        poolid = np.full((P, T * GW), 30000.0, dtype=np.float16)
        j = np.arange(n_real)
        for w in range(GW):
            poolid[j % P, (j // P) + w * T] = (bloc - w * P).astype(np.float32)
        d["poolid"] = poolid
        d["iota2"] = iota2

    rngs = np.where(rng_hi < 0, 0, rng_hi - np.maximum(rng_lo, 0))
    cfg = dict(T=T, NB=NB, GW=GW, N_loc=N_loc, n_cores=n_cores,
               band_tiles=band_tiles, band_lo=band_lo,
               CH=CH.tolist(), ch_off=ch_off.tolist(),
               col_off=col_off.tolist(),
               rng_lo=np.maximum(rng_lo, 0).tolist(),
               rng_hi=np.maximum(rng_hi, 0).tolist(),
               max_rng=int(rngs.max()), max_ch=int(CH.max()))
    return cfg, in_maps, bounds, g_lo, g_cnt


# ----------------------------------------------------------------------------
# Bass program
# ----------------------------------------------------------------------------

def _build_program(cfg):
    T, NB, GW, N_loc = cfg["T"], cfg["NB"], cfg["GW"], cfg["N_loc"]
    n_cores = cfg["n_cores"]
    band_tiles, band_lo = cfg["band_tiles"], cfg["band_lo"]
    CH, ch_off, col_off = cfg["CH"], cfg["ch_off"], cfg["col_off"]
    rng_lo, rng_hi = cfg["rng_lo"], cfg["rng_hi"]
    total_ch = ch_off[-1]
    D, DO = 128, 64

    nc = bacc.Bacc(None, num_devices=n_cores, num_swdge_queues=4)

    xT_d = nc.dram_tensor("xT", [P, N_loc], F32, kind="ExternalInput")
    W_d = {l: nc.dram_tensor(f"W{l}", [D, D], F32, kind="ExternalInput")
           for l in range(4)}
    b_d = {l: nc.dram_tensor(f"b{l}", [P, 1], F32, kind="ExternalInput")
           for l in range(4)}
    srcidx_d = nc.dram_tensor("srcidx", [P, total_ch * P // 16], I16,
                              kind="ExternalInput")
    dstid_d = nc.dram_tensor("dstid", [P, total_ch], F16,
                             kind="ExternalInput")
    disrep_d = nc.dram_tensor("disrep", [P, N_loc], F16, kind="ExternalInput")
    poolid_d = nc.dram_tensor("poolid", [P, T * GW], F16, kind="ExternalInput")
    iota2_d = nc.dram_tensor("iota2", [P, TB * P], F16, kind="ExternalInput")
    n_groups = NB * NB_BANDS
    gcnt_d = nc.dram_tensor("gcnt", [1, n_groups], I32, kind="ExternalInput")
    out_d = nc.dram_tensor("out", [GW * P, DO], F32, kind="ExternalOutput")
    dbg_stage = os.environ.get("GCN_DBG_STAGE", "")
    dbg_d = None
    if dbg_stage.startswith("h"):
        dbg_d = nc.dram_tensor("dbg", [P, N_loc], F32, kind="ExternalOutput")

    with tile.TileContext(nc) as tc:
        with tc.tile_pool(name="const", bufs=1) as const, \
             tc.tile_pool(name="hpool", bufs=1) as hpool, \
             tc.tile_pool(name="stage", bufs=3) as stage, \
             tc.tile_pool(name="rpool", bufs=2 * NB_BANDS) as rpool, \
             tc.tile_pool(name="spool", bufs=2) as spool, \
             tc.tile_pool(name="dram", bufs=2, space="DRAM") as dram, \
             tc.tile_pool(name="pm", bufs=2, space="PSUM") as pm, \
             tc.tile_pool(name="pq", bufs=1, space="PSUM") as pq, \
             tc.tile_pool(name="pt", bufs=2, space="PSUM") as pt, \
             tc.tile_pool(name="pa", bufs=2, space="PSUM") as pa:

            # ---- constants into SBUF
            w_sb = {}
            for l in range(4):
                w = const.tile([D, D], F32, name=f"w{l}sb")
                nc.sync.dma_start(out=w[:], in_=W_d[l][:, :])
                w_sb[l] = w
            b_sb = {}
            for l in range(4):
                b = const.tile([P, 1], F32, name=f"b{l}sb")
                nc.sync.dma_start(out=b[:], in_=b_d[l][:, :])
                b_sb[l] = b
            w16_sb = {}
            for l in range(1, 4):
                w16 = const.tile([D, D], F16, name=f"w16{l}sb")
                nc.scalar.copy(out=w16[:], in_=w_sb[l][:])
                w16_sb[l] = w16
            iota2_sb = const.tile([P, TB * P], F16, name="iota2sb")
            nc.sync.dma_start(out=iota2_sb[:], in_=iota2_d[:, :])
            iden16_sb = const.tile([P, P], F16, name="iden16sb")
            make_identity(nc, iden16_sb[:])
            srcidx_sb = const.tile([P, total_ch * P // 16], I16,
                                   name="srcidxsb")
            nc.sync.dma_start(out=srcidx_sb[:], in_=srcidx_d[:, :])
            dstid_sb = const.tile([P, total_ch], F16, name="dstidsb")
            disrep_sb = const.tile([P, N_loc], F16, name="disrepsb")
            poolid_sb = const.tile([P, T * GW], F16, name="poolidsb")
            nc.sync.dma_start(out=dstid_sb[:], in_=dstid_d[:, :])
            nc.sync.dma_start(out=disrep_sb[:], in_=disrep_d[:, :])
            nc.sync.dma_start(out=poolid_sb[:], in_=poolid_d[:, :])
            gcnt_sb = const.tile([1, n_groups], I32, name="gcntsb")
            nc.sync.dma_start(out=gcnt_sb[:], in_=gcnt_d[:, :])

            H = hpool.tile([P, N_loc], F16, name="H", tag="ha")
            Hsf = hpool.tile([P, N_loc], F16, name="Hsf", tag="hsf")

            # Zero the R pool once: trimmed (-1) gather slots are never
            # written by the DMA, and a NaN in untouched SBUF would poison
            # the scatter matmul (0 * NaN).  After this, stale slots only
            # ever hold old finite Hs values.
            for _ in range(2 * NB_BANDS):
                R0 = rpool.tile([P, cfg["max_ch"] * D], F16,
                                name="R", tag="R")
                nc.vector.memset(R0[:, :], 0.0)

            # Join const-load DMA sems into the DVE engine clock so later DVE
            # tensor_tensor ops don't carry per-DMA waits themselves.
            joiner = const.tile([P, 1], F32, name="joiner")
            for cst in [iota2_sb, dstid_sb, disrep_sb, poolid_sb,
                        b_sb[0], b_sb[1], b_sb[2], b_sb[3]]:
                nc.vector.tensor_copy(out=joiner[:, :1], in_=cst[:, :1])

            NKS = (N_loc + 511) // 512  # phase1 strips

            # ---- phase 1 strip: project cols [512k, 512k+512) of layer l.
            #      l=0 streams xT -> H; l>=1: Hsf = dis * (W^T H), transpose
            #      tiles into fp16 band tables.
            def phase1_strip(l, k, HsBands):
                c0 = k * 512
                cw = min(512, N_loc - c0)
                if l == 0:
                    xst = stage.tile([P, 512], F32, name="xst", tag="ms")
                    nc.sync.dma_start(out=xst[:, :cw],
                                      in_=xT_d[:, c0:c0 + cw])
                    mm = pm.tile([P, 512], F32, name="mm", tag="pm")
                    nc.tensor.matmul(mm[:, :cw], lhsT=w_sb[0][:, :],
                                     rhs=xst[:, :cw], start=True, stop=True)
                    nc.scalar.activation(
                        out=H[:, c0:c0 + cw], in_=mm[:, :cw],
                        func=AF.Identity, bias=b_sb[0][:, :], scale=1.0)
                    return
                mm = pm.tile([P, 512], F32, name="mm", tag="pm")
                nc.tensor.matmul(mm[:, :cw], lhsT=w16_sb[l][:, :],
                                 rhs=H[:, c0:c0 + cw], start=True, stop=True)
                nc.vector.tensor_tensor(
                    out=Hsf[:, c0:c0 + cw], in0=mm[:, :cw],
                    in1=disrep_sb[:, c0:c0 + cw], op=OP.mult)
                for tt in range(cw // P):
                    t = k * 4 + tt
                    j = 0
                    while t >= band_lo[j + 1]:
                        j += 1
                    trow = (t - band_lo[j]) * P
                    ptt = pt.tile([P, P], F16, name="ptt", tag="pt")
                    nc.tensor.transpose(
                        out=ptt[:, :],
                        in_=Hsf[:, t * P:(t + 1) * P],
                        identity=iden16_sb[:, :])
                    hs = stage.tile([P, P], F16, name="hs", tag="hs")
                    nc.scalar.copy(out=hs[:, :], in_=ptt[:, :])
                    nc.sync.dma_start(
                        out=HsBands[j][trow:trow + P, :],
                        in_=hs[:, :])

            def emit_cc(l, j, HsBands, HsFullBands):
                no_cc = int(os.environ.get("GCN_NO_CC", "0"))
                HsFull = dram.tile([n_cores * band_tiles[j] * P, D], F16,
                                   name=f"hsf{l}_{j}", tag=f"hsf{j}",
                                   addr_space="Local" if no_cc else "Shared")
                if no_cc:
                    nb_rows = band_tiles[j] * P
                    for cc_i in range(n_cores):
                        nc.sync.dma_start(
                            out=HsFull[cc_i * nb_rows:
                                       (cc_i + 1) * nb_rows, :],
                            in_=HsBands[j][:, :])
                else:
                    nc.gpsimd.collective_compute(
                        "AllGather", OP.bypass,
                        replica_groups=[list(range(n_cores))],
                        ins=[HsBands[j][:, :].opt()],
                        outs=[HsFull[:, :].opt()])
                HsFullBands.append(HsFull)

            def make_bands(l):
                return [dram.tile([band_tiles[j] * P, D], F16,
                                  name=f"hsl{l}_{j}", tag=f"hsl{j}")
                        for j in range(NB_BANDS)]

            # CC_j of the next layer fires after this phase1 strip:
            cc_strip = [(band_lo[j + 1] - 1) // 4 for j in range(NB_BANDS)]
            # phase1 strip s of the next layer is emitted after this block:
            strip_block = [min(2 * s + 1, NB - 1) for s in range(NKS)]

            # ---- phase 2: per block, gather 4 band groups (queues 0-3),
            #      then per tile scatter-matmul + local self term.  The next
            #      layer's phase1 strips + allgathers are interleaved so they
            #      hide under this layer's gather stream.
            def phase2(l, HsFullBands, H3, nxt):
                for tb in range(NB):
                    tbg = min(TB, T - tb * TB)
                    Rb = []
                    for j in range(NB_BANDS):
                        g = tb * NB_BANDS + j
                        nch = CH[g]
                        if nch == 0:
                            Rb.append(None)
                            continue
                        R = rpool.tile([P, cfg["max_ch"] * D], F16,
                                       name="R", tag="R")
                        num = nch * P
                        gcnt_reg = nc.gpsimd.alloc_register(f"gc{l}_{g}")
                        nc.gpsimd.reg_load(gcnt_reg, gcnt_sb[0:1, g:g + 1])
                        nc.gpsimd.dma_gather(
                            out_ap=R[:, :nch * D].rearrange(
                                "p (c e) -> p c e", e=D),
                            in_ap=HsFullBands[j][:, :],
                            idxs_ap=srcidx_sb[:, col_off[g]:
                                              col_off[g] + num // 16],
                            num_idxs=num,
                            num_idxs_reg=gcnt_reg,
                            elem_size=D,
                            single_packet=False,
                            queue_num=j)
                        Rb.append(R)
                    for tl in range(tbg):
                        phase2_tile(l, tb, tl, Rb, H3)
                    if nxt is not None:
                        for s in range(NKS):
                            if strip_block[s] == tb:
                                phase1_strip(l + 1, s, nxt[0])
                                for j in range(NB_BANDS):
                                    if cc_strip[j] == s:
                                        emit_cc(l + 1, j, nxt[0], nxt[1])

            def phase2_tile(l, tb, tl, Rb, H3):
                t = tb * TB + tl
                # selection matrices + accumulate matmuls over 4 bands
                mms = []
                for j in range(NB_BANDS):
                    g = tb * NB_BANDS + j
                    a, b2 = rng_lo[g][tl], rng_hi[g][tl]
                    if b2 <= a or Rb[j] is None:
                        continue
                    nr = b2 - a
                    S = spool.tile([P, cfg["max_rng"] * P], F16,
                                   name="S", tag="S")
                    nc.vector.tensor_tensor(
                        out=S[:, :nr * P].rearrange("p (c d) -> p c d", d=P),
                        in0=dstid_sb[:, ch_off[g] + a:ch_off[g] + b2]
                            .unsqueeze(2).broadcast_to([P, nr, P]),
                        in1=iota2_sb[:, tl * P:(tl + 1) * P]
                            .unsqueeze(1).broadcast_to([P, nr, P]),
                        op=OP.is_equal)
                    for cc in range(a, b2):
                        mms.append((Rb[j], cc, S, cc - a))
                agg = pa.tile([P, P], F32, name="agg", tag="pa")
                if not mms:
                    nc.vector.memset(agg[:, :], 0.0)
                for k, (R, cc, S, sc) in enumerate(mms):
                    nc.tensor.matmul(
                        agg[:, :],
                        lhsT=R[:, cc * D:(cc + 1) * D],
                        rhs=S[:, sc * P:(sc + 1) * P],
                        start=(k == 0), stop=(k == len(mms) - 1))
                # out = dis_d * (agg + Hsf_d) + b ; relu except l=3
                tmp = stage.tile([P, P], F32, name="tmp", tag="tmp")
                nc.vector.tensor_tensor(
                    out=tmp[:, :], in0=agg[:, :],
                    in1=Hsf[:, t * P:(t + 1) * P], op=OP.add)
                nc.vector.tensor_tensor(
                    out=tmp[:, :], in0=tmp[:, :],
                    in1=disrep_sb[:, t * P:(t + 1) * P], op=OP.mult)
                if l < 3:
                    nc.scalar.activation(
                        out=H[:, t * P:(t + 1) * P], in_=tmp[:, :],
                        func=AF.Relu, bias=b_sb[l][:, :], scale=1.0)
                else:
                    t2 = stage.tile([P, P], F16, name="t2", tag="tmp2")
                    nc.scalar.activation(
                        out=t2[:, :], in_=tmp[:, :],
                        func=AF.Identity, bias=b_sb[3][:, :], scale=1.0)
                    ptt = pt.tile([P, P], F16, name="ptt2", tag="pt")
                    nc.tensor.transpose(
                        out=ptt[:, :], in_=t2[:, :],
                        identity=iden16_sb[:, :])
                    nc.scalar.copy(
                        out=H3[:, t * DO:(t + 1) * DO],
                        in_=ptt[:, :DO])
                    if GW == 1:
                        sp = spool.tile([P, P], F16, name="sp", tag="sp")
                        nc.vector.tensor_tensor(
                            out=sp[:],
                            in0=poolid_sb[:, t:t + 1].to_broadcast([P, P]),
                            in1=iota2_sb[:, :P], op=OP.is_equal)
                        nc.tensor.matmul(pp_hold[0][:], lhsT=sp[:],
                                         rhs=H3[:, t * DO:(t + 1) * DO],
                                         start=(t == 0), stop=(t == T - 1))

            def dump_dbg(buf, width=None):
                nc.sync.dma_start(
                    out=dbg_d[:, :width] if width else dbg_d[:, :],
                    in_=buf[:, :width] if width else buf[:, :])

            # ---- the network
            H3 = hpool.tile([P, T * DO], F16, name="H3", tag="hx")
            for k in range(NKS):
                phase1_strip(0, k, None)  # embedding -> H (streams xT)
            if dbg_stage == "h0":
                dump_dbg(H)
            bands1 = make_bands(1)
            full1 = []
            for k in range(NKS):
                phase1_strip(1, k, bands1)
                for j in range(NB_BANDS):
                    if cc_strip[j] == k:
                        emit_cc(1, j, bands1, full1)
            ctx = (bands1, full1)
            pp_hold = []
            for l in (1, 2, 3):
                if l == 3 and GW == 1:
                    pp_hold.append(pq.tile([P, DO], F32, name="pp", tag="pp"))
                if l < 3:
                    nxt = (make_bands(l + 1), [])
                else:
                    nxt = None
                phase2(l, ctx[1], H3, nxt)
                if dbg_stage == f"h{l}":
                    dump_dbg(H if l < 3 else H3, None if l < 3 else T * DO)
                ctx = nxt

            # ---- global add pool (accumulated inline during layer 3 when
            #      GW == 1; fall back to a tail loop otherwise)
            if GW == 1:
                ost = stage.tile([P, DO], F32, name="ost", tag="ost")
                nc.scalar.copy(out=ost[:], in_=pp_hold[0][:])
                nc.sync.dma_start(out=out_d[0:P, :], in_=ost[:])
            else:
                for w in range(GW):
                    pp = pt.tile([P, DO], F32, name="pp", tag="pp")
                    for t in range(T):
                        sp = spool.tile([P, P], F16, name="sp", tag="sp")
                        nc.vector.tensor_tensor(
                            out=sp[:],
                            in0=poolid_sb[:, w * T + t:w * T + t + 1]
                                .to_broadcast([P, P]),
                            in1=iota2_sb[:, :P], op=OP.is_equal)
                        nc.tensor.matmul(pp[:], lhsT=sp[:],
                                         rhs=H3[:, t * DO:(t + 1) * DO],
                                         start=(t == 0), stop=(t == T - 1))
                    ost = stage.tile([P, DO], F32, name="ost", tag="ost")
                    nc.scalar.copy(out=ost[:], in_=pp[:])
                    nc.sync.dma_start(out=out_d[w * P:(w + 1) * P, :],
                                      in_=ost[:])

    return nc


# ----------------------------------------------------------------------------
# Driver
# ----------------------------------------------------------------------------

def _run(x, edge_index, batch, W_emb, b_emb, W1, b1, W2, b2, W3, b3,
         G=G_TOTAL, n_cores=N_CORES, trace=False):
    x = np.ascontiguousarray(np.asarray(x, dtype=np.float32))
    edge_index = np.ascontiguousarray(np.asarray(edge_index, dtype=np.int64))
    batch_np = np.ascontiguousarray(np.asarray(batch, dtype=np.int64))

    cfg, in_maps, bounds, g_lo, g_cnt = _preprocess(
        x, edge_index, batch_np, n_cores, G)

    def bpad(b):
        v = np.zeros((P, 1), dtype=np.float32)
        b = np.asarray(b, dtype=np.float32).reshape(-1)
        v[:b.shape[0], 0] = b
        return v

    W3p = np.zeros((128, 128), dtype=np.float32)
    W3p[:, :np.asarray(W3).shape[1]] = np.asarray(W3, dtype=np.float32)
    shared = dict(
        W0=np.asarray(W_emb, dtype=np.float32),
        W1=np.asarray(W1, dtype=np.float32),
        W2=np.asarray(W2, dtype=np.float32),
        W3=W3p,
        b0=bpad(b_emb), b1=bpad(b1), b2=bpad(b2), b3=bpad(b3))
    for m in in_maps:
        m.update(shared)

    nc = _build_program(cfg)
    nc.finalize()
    res = run_bass_kernel_spmd(nc, in_maps, list(range(n_cores)),
                               trace=trace)

    out = np.zeros((G, 64), dtype=np.float32)
    for c in range(n_cores):
        oc = np.asarray(res.results[c]["out"])
        if g_cnt[c] > 0:
            out[g_lo[c]:g_lo[c] + g_cnt[c]] = oc[:g_cnt[c]]
    return out, res


def kernel(**inputs):
    out, _ = _run(G=G_TOTAL, n_cores=N_CORES,
                  trace=bool(int(os.environ.get("GCN_TRACE", "0"))),
                  **inputs)
    return out


# revision 27
# speedup vs baseline: 1.7275x; 1.0133x over previous
"""Trainium2 Bass kernel for a 3-layer GCN (Kipf-Welling, symmetric norm,
self-loops) with global add pooling.

Distribution: nodes (graph-aligned contiguous ranges) are sharded across 8
NeuronCores.  Each core owns the aggregation (scatter-add) for its local dst
nodes.  Per layer the (dis-scaled) projected features Hs are exchanged in 4
tile-BANDS via 4 pipelined AllGathers; band j's edge gathers run on SWDGE
queue j so desc-gen pipelines across the 4 Q7 cpu pairs.

Math (matches the jax reference exactly):
    deg  = indeg + 1, dis = deg^-1/2
    Hsf  = dis * (H @ W)                      (feature-major, kept in SBUF)
    agg_d = dis_d * (sum_{e:(s->d)} Hsf_s  +  Hsf_d) + b     (self loop local)
    H'   = relu(agg)                          (no relu on layer 3)
    out  = segment_sum(H3, batch)

The edge scatter-add is a matmul with an on-the-fly selection matrix
S[e, d] = (dst_id[e] == d) in fp16; gathered source rows are fp16.
Layer 3 is zero-padded to 128 output features so fp16 table rows stay 256B
(dma_gather elem_size restriction).
"""

import os
import sys
import math

import numpy as np

sys.path.insert(0, "/opt/trn_rl_repo")

import concourse.bass as bass  # noqa: E402
import concourse.bacc as bacc  # noqa: E402
import concourse.tile as tile  # noqa: E402
from concourse import mybir  # noqa: E402
from concourse.bass_utils import run_bass_kernel_spmd  # noqa: E402
from concourse.masks import make_identity  # noqa: E402

P = 128
F32 = mybir.dt.float32
F16 = mybir.dt.float16
I32 = mybir.dt.int32
I16 = mybir.dt.int16
OP = mybir.AluOpType
AF = mybir.ActivationFunctionType

N_CORES = 8
G_TOTAL = 1000  # graphs in the batch (fixed by the problem)
NB_BANDS = 4    # src-tile bands (= SWDGE queues = pipelined allgathers)
TB = 2          # dst tiles per gather block


# ----------------------------------------------------------------------------
# Host-side preprocessing: shard nodes/edges, build gather/selection metadata.
# ----------------------------------------------------------------------------

def _preprocess(x, edge_index, batch, n_cores, G):
    N = x.shape[0]
    src = edge_index[0].astype(np.int64)
    dst = edge_index[1].astype(np.int64)
    batch = batch.astype(np.int64)

    # graph-aligned shard boundaries near equal node counts
    graph_start = np.searchsorted(batch, np.arange(G + 1))  # [G+1], node idx
    bounds = [0]
    for c in range(1, n_cores):
        target = (c * N) // n_cores
        gi = np.searchsorted(graph_start, target)
        lo = graph_start[gi - 1] if gi > 0 else 0
        hi = graph_start[gi] if gi <= G else N
        b = int(hi if (hi - target) <= (target - lo) else lo)
        b = max(b, bounds[-1])  # keep non-decreasing
        bounds.append(b)
    bounds.append(N)
    bounds = np.asarray(bounds, dtype=np.int64)

    shard_sizes = bounds[1:] - bounds[:-1]
    N_loc = int(math.ceil(int(shard_sizes.max()) / P) * P)
    T = N_loc // P
    NB = (T + TB - 1) // TB

    # band boundaries over local tiles
    bs = (T + NB_BANDS - 1) // NB_BANDS
    band_lo = [min(j * bs, T) for j in range(NB_BANDS + 1)]
    band_tiles = [band_lo[j + 1] - band_lo[j] for j in range(NB_BANDS)]

    # normalization (index-derived scalars)
    deg = np.bincount(dst, minlength=N).astype(np.float32) + np.float32(1.0)
    dis = (np.float32(1.0) / np.sqrt(deg)).astype(np.float32)

    # src row id within its band table:
    # band table j rows: (src_core * band_tiles[j] + tile_in_band)*128 + pos
    core_of = np.searchsorted(bounds, dst, side="right") - 1
    core_of_src = np.searchsorted(bounds, src, side="right") - 1
    src_loc = src - bounds[core_of_src]
    src_tile = src_loc // P
    src_band = np.minimum(src_tile // bs, NB_BANDS - 1)
    src_row = (core_of_src * np.asarray(band_tiles)[src_band]
               + (src_tile - np.asarray(band_lo)[src_band])) * P + src_loc % P
    assert int(src_row.max(initial=0)) < 32768

    # ---- per-core edge bucketing by (dst block, src band), sorted by dst
    per_core = []
    for c in range(n_cores):
        m = core_of == c
        dstl = dst[m] - bounds[c]
        rows = src_row[m]
        bands = src_band[m]
        blk = dstl // (TB * P)
        key = blk * NB_BANDS + bands
        order = np.lexsort((dstl, key))
        dstl, rows, key = dstl[order], rows[order], key[order]
        cnt = np.bincount(key, minlength=NB * NB_BANDS)
        per_core.append((dstl, rows, key, cnt))

    cnts = np.stack([pc[3] for pc in per_core])        # [cores, NB*NB_BANDS]
    CH = np.ceil(cnts.max(axis=0) / P).astype(np.int64)  # chunks per group
    ch_off = np.concatenate([[0], np.cumsum(CH)])      # chunk col offsets
    total_ch = int(ch_off[-1])
    # idx16 column offset per group (16-wrapped, so 8 cols per chunk)
    col_off = ch_off * (P // 16)

    # per-(group, tile-in-block) chunk subranges, unioned over cores
    rng_lo = np.full((NB * NB_BANDS, TB), 10 ** 9, dtype=np.int64)
    rng_hi = np.full((NB * NB_BANDS, TB), -1, dtype=np.int64)

    in_maps = []
    g_lo = []
    g_cnt = []
    for c in range(n_cores):
        dstl, rows, key, cnt = per_core[c]
        goff = np.concatenate([[0], np.cumsum(cnt)])[:-1]
        slot = (np.arange(dstl.shape[0]) - goff[key]) + ch_off[key] * P

        # trailing -1 pads are trimmed by the Q7 firmware; num_idxs_reg
        # carries the matching per-core count so the decode-side ring
        # reservation agrees with what the firmware pushes.
        idx16 = np.full((16, total_ch * P // 16), -1, dtype=np.int16)
        idx16[slot % 16, slot // 16] = rows.astype(np.int16)
        srcidx = np.tile(idx16, (8, 1))

        dstid = np.full((P, total_ch), 30000.0, dtype=np.float16)
        dstid[slot % P, slot // P] = (dstl % (TB * P)).astype(np.float32)

        # per-core tile chunk ranges -> union
        tl = (dstl // P) % TB
        gkey = key * TB + tl
        gcnt = np.bincount(gkey, minlength=NB * NB_BANDS * TB)
        goff2 = np.concatenate([[0], np.cumsum(gcnt)])
        for g in range(NB * NB_BANDS):
            for t2 in range(TB):
                s0, s1 = goff2[g * TB + t2], goff2[g * TB + t2 + 1]
                if s1 > s0:
                    a = (s0 - goff[g]) // P
                    b = (s1 - 1 - goff[g]) // P + 1
                    rng_lo[g, t2] = min(rng_lo[g, t2], a)
                    rng_hi[g, t2] = max(rng_hi[g, t2], b)

        n_real = int(bounds[c + 1] - bounds[c])
        dis_loc = np.ones(N_loc, dtype=np.float32)
        dis_loc[:n_real] = dis[bounds[c]:bounds[c + 1]]
        disrep = np.broadcast_to(dis_loc.astype(np.float16), (P, N_loc)).copy()

        xT = np.zeros((P, N_loc), dtype=np.float32)
        xT[:, :n_real] = x[bounds[c]:bounds[c + 1]].T

        bloc = batch[bounds[c]:bounds[c + 1]]
        glo = int(bloc[0]) if n_real > 0 else 0
        gct = int(bloc[-1]) + 1 - glo if n_real > 0 else 0
        g_lo.append(glo)
        g_cnt.append(gct)
        gcnt = cnt.astype(np.int32).reshape(1, -1)
        in_maps.append(dict(srcidx=srcidx, dstid=dstid, disrep=disrep, xT=xT,
                            gcnt=gcnt, _bloc=bloc - glo, _n_real=n_real))

    GW = max(1, int(math.ceil(max(g_cnt) / P)))
    iota2 = np.zeros((P, TB * P), dtype=np.float16)
    for t2 in range(TB):
        iota2[:, t2 * P:(t2 + 1) * P] = np.arange(P) + t2 * P
    for c in range(n_cores):
        d = in_maps[c]
        bloc, n_real = d.pop("_bloc"), d.pop("_n_real")
        poolid = np.full((P, T * GW), 30000.0, dtype=np.float16)
        j = np.arange(n_real)
        for w in range(GW):
            poolid[j % P, (j // P) + w * T] = (bloc - w * P).astype(np.float32)
        d["poolid"] = poolid
        d["iota2"] = iota2

    rngs = np.where(rng_hi < 0, 0, rng_hi - np.maximum(rng_lo, 0))
    cfg = dict(T=T, NB=NB, GW=GW, N_loc=N_loc, n_cores=n_cores,
               band_tiles=band_tiles, band_lo=band_lo,
               CH=CH.tolist(), ch_off=ch_off.tolist(),
               col_off=col_off.tolist(),
               rng_lo=np.maximum(rng_lo, 0).tolist(),
               rng_hi=np.maximum(rng_hi, 0).tolist(),
               max_rng=int(rngs.max()), max_ch=int(CH.max()))
    return cfg, in_maps, bounds, g_lo, g_cnt


# ----------------------------------------------------------------------------
# Bass program
# ----------------------------------------------------------------------------

def _build_program(cfg):
    T, NB, GW, N_loc = cfg["T"], cfg["NB"], cfg["GW"], cfg["N_loc"]
    n_cores = cfg["n_cores"]
    band_tiles, band_lo = cfg["band_tiles"], cfg["band_lo"]
    CH, ch_off, col_off = cfg["CH"], cfg["ch_off"], cfg["col_off"]
    rng_lo, rng_hi = cfg["rng_lo"], cfg["rng_hi"]
    total_ch = ch_off[-1]
    D, DO = 128, 64

    nc = bacc.Bacc(None, num_devices=n_cores, num_swdge_queues=4)

    xT_d = nc.dram_tensor("xT", [P, N_loc], F32, kind="ExternalInput")
    W_d = {l: nc.dram_tensor(f"W{l}", [D, D], F32, kind="ExternalInput")
           for l in range(4)}
    b_d = {l: nc.dram_tensor(f"b{l}", [P, 1], F32, kind="ExternalInput")
           for l in range(4)}
    srcidx_d = nc.dram_tensor("srcidx", [P, total_ch * P // 16], I16,
                              kind="ExternalInput")
    dstid_d = nc.dram_tensor("dstid", [P, total_ch], F16,
                             kind="ExternalInput")
    disrep_d = nc.dram_tensor("disrep", [P, N_loc], F16, kind="ExternalInput")
    poolid_d = nc.dram_tensor("poolid", [P, T * GW], F16, kind="ExternalInput")
    iota2_d = nc.dram_tensor("iota2", [P, TB * P], F16, kind="ExternalInput")
    n_groups = NB * NB_BANDS
    gcnt_d = nc.dram_tensor("gcnt", [1, n_groups], I32, kind="ExternalInput")
    out_d = nc.dram_tensor("out", [GW * P, DO], F32, kind="ExternalOutput")
    dbg_stage = os.environ.get("GCN_DBG_STAGE", "")
    dbg_d = None
    if dbg_stage.startswith("h"):
        dbg_d = nc.dram_tensor("dbg", [P, N_loc], F32, kind="ExternalOutput")

    with tile.TileContext(nc) as tc:
        with tc.tile_pool(name="const", bufs=1) as const, \
             tc.tile_pool(name="hpool", bufs=1) as hpool, \
             tc.tile_pool(name="stage", bufs=3) as stage, \
             tc.tile_pool(name="rpool", bufs=2 * NB_BANDS) as rpool, \
             tc.tile_pool(name="spool", bufs=2) as spool, \
             tc.tile_pool(name="dram", bufs=2, space="DRAM") as dram, \
             tc.tile_pool(name="pm", bufs=2, space="PSUM") as pm, \
             tc.tile_pool(name="pq", bufs=1, space="PSUM") as pq, \
             tc.tile_pool(name="pt", bufs=2, space="PSUM") as pt, \
             tc.tile_pool(name="pa", bufs=2, space="PSUM") as pa:

            # ---- constants into SBUF
            w_sb = {}
            for l in range(4):
                w = const.tile([D, D], F32, name=f"w{l}sb")
                nc.sync.dma_start(out=w[:], in_=W_d[l][:, :])
                w_sb[l] = w
            b_sb = {}
            for l in range(4):
                b = const.tile([P, 1], F32, name=f"b{l}sb")
                nc.sync.dma_start(out=b[:], in_=b_d[l][:, :])
                b_sb[l] = b
            w16_sb = {}
            for l in range(1, 4):
                w16 = const.tile([D, D], F16, name=f"w16{l}sb")
                nc.scalar.copy(out=w16[:], in_=w_sb[l][:])
                w16_sb[l] = w16
            iota2_sb = const.tile([P, TB * P], F16, name="iota2sb")
            nc.sync.dma_start(out=iota2_sb[:], in_=iota2_d[:, :])
            iden16_sb = const.tile([P, P], F16, name="iden16sb")
            make_identity(nc, iden16_sb[:])
            srcidx_sb = const.tile([P, total_ch * P // 16], I16,
                                   name="srcidxsb")
            nc.sync.dma_start(out=srcidx_sb[:], in_=srcidx_d[:, :])
            dstid_sb = const.tile([P, total_ch], F16, name="dstidsb")
            disrep_sb = const.tile([P, N_loc], F16, name="disrepsb")
            poolid_sb = const.tile([P, T * GW], F16, name="poolidsb")
            nc.sync.dma_start(out=dstid_sb[:], in_=dstid_d[:, :])
            nc.sync.dma_start(out=disrep_sb[:], in_=disrep_d[:, :])
            nc.sync.dma_start(out=poolid_sb[:], in_=poolid_d[:, :])
            gcnt_sb = const.tile([1, n_groups], I32, name="gcntsb")
            nc.sync.dma_start(out=gcnt_sb[:], in_=gcnt_d[:, :])

            H = hpool.tile([P, N_loc], F16, name="H", tag="ha")
            Hsf = hpool.tile([P, N_loc], F16, name="Hsf", tag="hsf")

            # Zero the R pool once: trimmed (-1) gather slots are never
            # written by the DMA, and a NaN in untouched SBUF would poison
            # the scatter matmul (0 * NaN).  After this, stale slots only
            # ever hold old finite Hs values.
            for _ in range(2 * NB_BANDS):
                R0 = rpool.tile([P, cfg["max_ch"] * D], F16,
                                name="R", tag="R")
                nc.vector.memset(R0[:, :], 0.0)

            # Join const-load DMA sems into the DVE engine clock so later DVE
            # tensor_tensor ops don't carry per-DMA waits themselves.
            joiner = const.tile([P, 1], F32, name="joiner")
            for cst in [iota2_sb, dstid_sb, disrep_sb, poolid_sb,
                        b_sb[0], b_sb[1], b_sb[2], b_sb[3]]:
                nc.vector.tensor_copy(out=joiner[:, :1], in_=cst[:, :1])

            NKS = (N_loc + 511) // 512  # phase1 strips

            # ---- phase 1 strip: project cols [512k, 512k+512) of layer l.
            #      l=0 streams xT -> H; l>=1: Hsf = dis * (W^T H), transpose
            #      tiles into fp16 band tables.
            def phase1_strip(l, k, HsBands):
                c0 = k * 512
                cw = min(512, N_loc - c0)
                if l == 0:
                    xst = stage.tile([P, 512], F32, name="xst", tag="ms")
                    nc.sync.dma_start(out=xst[:, :cw],
                                      in_=xT_d[:, c0:c0 + cw])
                    mm = pm.tile([P, 512], F32, name="mm", tag="pm")
                    nc.tensor.matmul(mm[:, :cw], lhsT=w_sb[0][:, :],
                                     rhs=xst[:, :cw], start=True, stop=True)
                    nc.scalar.activation(
                        out=H[:, c0:c0 + cw], in_=mm[:, :cw],
                        func=AF.Identity, bias=b_sb[0][:, :], scale=1.0)
                    return
                mm = pm.tile([P, 512], F32, name="mm", tag="pm")
                nc.tensor.matmul(mm[:, :cw], lhsT=w16_sb[l][:, :],
                                 rhs=H[:, c0:c0 + cw], start=True, stop=True)
                nc.vector.tensor_tensor(
                    out=Hsf[:, c0:c0 + cw], in0=mm[:, :cw],
                    in1=disrep_sb[:, c0:c0 + cw], op=OP.mult)
                for tt in range(cw // P):
                    t = k * 4 + tt
                    j = 0
                    while t >= band_lo[j + 1]:
                        j += 1
                    trow = (t - band_lo[j]) * P
                    ptt = pt.tile([P, P], F16, name="ptt", tag="pt")
                    nc.tensor.transpose(
                        out=ptt[:, :],
                        in_=Hsf[:, t * P:(t + 1) * P],
                        identity=iden16_sb[:, :])
                    hs = stage.tile([P, P], F16, name="hs", tag="hs")
                    nc.scalar.copy(out=hs[:, :], in_=ptt[:, :])
                    nc.sync.dma_start(
                        out=HsBands[j][trow:trow + P, :],
                        in_=hs[:, :])

            def emit_cc(l, j, HsBands, HsFullBands):
                no_cc = int(os.environ.get("GCN_NO_CC", "0"))
                HsFull = dram.tile([n_cores * band_tiles[j] * P, D], F16,
                                   name=f"hsf{l}_{j}", tag=f"hsf{j}",
                                   addr_space="Local" if no_cc else "Shared")
                if no_cc:
                    nb_rows = band_tiles[j] * P
                    for cc_i in range(n_cores):
                        nc.sync.dma_start(
                            out=HsFull[cc_i * nb_rows:
                                       (cc_i + 1) * nb_rows, :],
                            in_=HsBands[j][:, :])
                else:
                    nc.gpsimd.collective_compute(
                        "AllGather", OP.bypass,
                        replica_groups=[list(range(n_cores))],
                        ins=[HsBands[j][:, :].opt()],
                        outs=[HsFull[:, :].opt()])
                HsFullBands.append(HsFull)

            def make_bands(l):
                return [dram.tile([band_tiles[j] * P, D], F16,
                                  name=f"hsl{l}_{j}", tag=f"hsl{j}")
                        for j in range(NB_BANDS)]

            # CC_j of the next layer fires after this phase1 strip:
            cc_strip = [(band_lo[j + 1] - 1) // 4 for j in range(NB_BANDS)]
            # phase1 strip s of the next layer is emitted after this block:
            strip_block = [min(2 * s + 1, NB - 1) for s in range(NKS)]

            # ---- phase 2: per block, gather 4 band groups (queues 0-3),
            #      then per tile scatter-matmul + local self term.  The next
            #      layer's phase1 strips + allgathers are interleaved so they
            #      hide under this layer's gather stream.
            def phase2(l, HsFullBands, H3, nxt):
                for tb in range(NB):
                    tbg = min(TB, T - tb * TB)
                    Rb = []
                    for j in range(NB_BANDS):
                        g = tb * NB_BANDS + j
                        nch = CH[g]
                        if nch == 0:
                            Rb.append(None)
                            continue
                        R = rpool.tile([P, cfg["max_ch"] * D], F16,
                                       name="R", tag="R")
                        num = nch * P
                        gcnt_reg = nc.gpsimd.alloc_register(f"gc{l}_{g}")
                        nc.gpsimd.reg_load(gcnt_reg, gcnt_sb[0:1, g:g + 1])
                        nc.gpsimd.dma_gather(
                            out_ap=R[:, :nch * D].rearrange(
                                "p (c e) -> p c e", e=D),
                            in_ap=HsFullBands[j][:, :],
                            idxs_ap=srcidx_sb[:, col_off[g]:
                                              col_off[g] + num // 16],
                            num_idxs=num,
                            num_idxs_reg=gcnt_reg,
                            elem_size=D,
                            single_packet=False,
                            queue_num=j)
                        Rb.append(R)
                    for tl in range(tbg):
                        phase2_tile(l, tb, tl, Rb, H3)
                    if nxt is not None:
                        for s in range(NKS):
                            if strip_block[s] == tb:
                                phase1_strip(l + 1, s, nxt[0])
                                for j in range(NB_BANDS):
                                    if cc_strip[j] == s:
                                        emit_cc(l + 1, j, nxt[0], nxt[1])

            def phase2_tile(l, tb, tl, Rb, H3):
                t = tb * TB + tl
                # selection matrices + accumulate matmuls over 4 bands
                mms = []
                for j in range(NB_BANDS):
                    g = tb * NB_BANDS + j
                    a, b2 = rng_lo[g][tl], rng_hi[g][tl]
                    if b2 <= a or Rb[j] is None:
                        continue
                    nr = b2 - a
                    S = spool.tile([P, cfg["max_rng"] * P], F16,
                                   name="S", tag="S")
                    nc.vector.tensor_tensor(
                        out=S[:, :nr * P].rearrange("p (c d) -> p c d", d=P),
                        in0=dstid_sb[:, ch_off[g] + a:ch_off[g] + b2]
                            .unsqueeze(2).broadcast_to([P, nr, P]),
                        in1=iota2_sb[:, tl * P:(tl + 1) * P]
                            .unsqueeze(1).broadcast_to([P, nr, P]),
                        op=OP.is_equal)
                    for cc in range(a, b2):
                        mms.append((Rb[j], cc, S, cc - a))
                agg = pa.tile([P, P], F32, name="agg", tag="pa")
                if not mms:
                    nc.vector.memset(agg[:, :], 0.0)
                for k, (R, cc, S, sc) in enumerate(mms):
                    nc.tensor.matmul(
                        agg[:, :],
                        lhsT=R[:, cc * D:(cc + 1) * D],
                        rhs=S[:, sc * P:(sc + 1) * P],
                        start=(k == 0), stop=(k == len(mms) - 1))
                # out = dis_d * (agg + Hsf_d) + b ; relu except l=3
                tmp = stage.tile([P, P], F32, name="tmp", tag="tmp")
                nc.vector.tensor_tensor(
                    out=tmp[:, :], in0=agg[:, :],
                    in1=Hsf[:, t * P:(t + 1) * P], op=OP.add)
                nc.vector.tensor_tensor(
                    out=tmp[:, :], in0=tmp[:, :],
                    in1=disrep_sb[:, t * P:(t + 1) * P], op=OP.mult)
                if l < 3:
                    nc.scalar.activation(
                        out=H[:, t * P:(t + 1) * P], in_=tmp[:, :],
                        func=AF.Relu, bias=b_sb[l][:, :], scale=1.0)
                else:
                    t2 = stage.tile([P, P], F16, name="t2", tag="tmp2")
                    nc.scalar.activation(
                        out=t2[:, :], in_=tmp[:, :],
                        func=AF.Identity, bias=b_sb[3][:, :], scale=1.0)
                    ptt = pt.tile([P, P], F16, name="ptt2", tag="pt")
                    nc.tensor.transpose(
                        out=ptt[:, :], in_=t2[:, :],
                        identity=iden16_sb[:, :])
                    nc.scalar.copy(
                        out=H3[:, t * DO:(t + 1) * DO],
                        in_=ptt[:, :DO])
                    if GW == 1:
                        sp = spool.tile([P, P], F16, name="sp", tag="sp")
                        nc.vector.tensor_tensor(
                            out=sp[:],
                            in0=poolid_sb[:, t:t + 1].to_broadcast([P, P]),
                            in1=iota2_sb[:, :P], op=OP.is_equal)
                        nc.tensor.matmul(pp_hold[0][:], lhsT=sp[:],
                                         rhs=H3[:, t * DO:(t + 1) * DO],
                                         start=(t == 0), stop=(t == T - 1))

            def dump_dbg(buf, width=None):
                nc.sync.dma_start(
                    out=dbg_d[:, :width] if width else dbg_d[:, :],
                    in_=buf[:, :width] if width else buf[:, :])

            # ---- the network
            H3 = hpool.tile([P, T * DO], F16, name="H3", tag="hx")
            bands1 = make_bands(1)
            full1 = []
            for k in range(NKS):
                phase1_strip(0, k, None)  # embedding -> H (streams xT)
                phase1_strip(1, k, bands1)
                for j in range(NB_BANDS):
                    if cc_strip[j] == k:
                        emit_cc(1, j, bands1, full1)
            if dbg_stage == "h0":
                dump_dbg(H)
            ctx = (bands1, full1)
            pp_hold = []
            for l in (1, 2, 3):
                if l == 3 and GW == 1:
                    pp_hold.append(pq.tile([P, DO], F32, name="pp", tag="pp"))
                if l < 3:
                    nxt = (make_bands(l + 1), [])
                else:
                    nxt = None
                phase2(l, ctx[1], H3, nxt)
                if dbg_stage == f"h{l}":
                    dump_dbg(H if l < 3 else H3, None if l < 3 else T * DO)
                ctx = nxt

            # ---- global add pool (accumulated inline during layer 3 when
            #      GW == 1; fall back to a tail loop otherwise)
            if GW == 1:
                ost = stage.tile([P, DO], F32, name="ost", tag="ost")
                nc.scalar.copy(out=ost[:], in_=pp_hold[0][:])
                nc.sync.dma_start(out=out_d[0:P, :], in_=ost[:])
            else:
                for w in range(GW):
                    pp = pt.tile([P, DO], F32, name="pp", tag="pp")
                    for t in range(T):
                        sp = spool.tile([P, P], F16, name="sp", tag="sp")
                        nc.vector.tensor_tensor(
                            out=sp[:],
                            in0=poolid_sb[:, w * T + t:w * T + t + 1]
                                .to_broadcast([P, P]),
                            in1=iota2_sb[:, :P], op=OP.is_equal)
                        nc.tensor.matmul(pp[:], lhsT=sp[:],
                                         rhs=H3[:, t * DO:(t + 1) * DO],
                                         start=(t == 0), stop=(t == T - 1))
                    ost = stage.tile([P, DO], F32, name="ost", tag="ost")
                    nc.scalar.copy(out=ost[:], in_=pp[:])
                    nc.sync.dma_start(out=out_d[w * P:(w + 1) * P, :],
                                      in_=ost[:])

    return nc


# ----------------------------------------------------------------------------
# Driver
# ----------------------------------------------------------------------------

def _run(x, edge_index, batch, W_emb, b_emb, W1, b1, W2, b2, W3, b3,
         G=G_TOTAL, n_cores=N_CORES, trace=False):
    x = np.ascontiguousarray(np.asarray(x, dtype=np.float32))
    edge_index = np.ascontiguousarray(np.asarray(edge_index, dtype=np.int64))
    batch_np = np.ascontiguousarray(np.asarray(batch, dtype=np.int64))

    cfg, in_maps, bounds, g_lo, g_cnt = _preprocess(
        x, edge_index, batch_np, n_cores, G)

    def bpad(b):
        v = np.zeros((P, 1), dtype=np.float32)
        b = np.asarray(b, dtype=np.float32).reshape(-1)
        v[:b.shape[0], 0] = b
        return v

    W3p = np.zeros((128, 128), dtype=np.float32)
    W3p[:, :np.asarray(W3).shape[1]] = np.asarray(W3, dtype=np.float32)
    shared = dict(
        W0=np.asarray(W_emb, dtype=np.float32),
        W1=np.asarray(W1, dtype=np.float32),
        W2=np.asarray(W2, dtype=np.float32),
        W3=W3p,
        b0=bpad(b_emb), b1=bpad(b1), b2=bpad(b2), b3=bpad(b3))
    for m in in_maps:
        m.update(shared)

    nc = _build_program(cfg)
    nc.finalize()
    res = run_bass_kernel_spmd(nc, in_maps, list(range(n_cores)),
                               trace=trace)

    out = np.zeros((G, 64), dtype=np.float32)
    for c in range(n_cores):
        oc = np.asarray(res.results[c]["out"])
        if g_cnt[c] > 0:
            out[g_lo[c]:g_lo[c] + g_cnt[c]] = oc[:g_cnt[c]]
    return out, res


def kernel(**inputs):
    out, _ = _run(G=G_TOTAL, n_cores=N_CORES,
                  trace=bool(int(os.environ.get("GCN_TRACE", "0"))),
                  **inputs)
    return out
